# revision 32
# baseline (speedup 1.0000x reference)
"""Trainium2 Bass kernel for nn_Decoder (LSTM decoder with residual output feedback).

Model (per batch row):
    h0 = c0 = z @ W_proj.T + b_proj                      # [B, H]
    y0 = x[:, -1, :]                                     # [B, X]
    per step t: gates = y_{t-1} @ W_ih.T + h_{t-1} @ W_hh.T + (b_ih + b_hh)
                i, f, g, o = split(gates); c = sig(f)*c + sig(i)*tanh(g)
                h = sig(o)*tanh(c); y_t = y_{t-1} + h @ W_out.T + b_out
    out = stack(y_1..y_T)                                # [B, T, Y]

Strategy:
  * Pure data-parallel over batch: B=1024 -> 128 rows/core on 8 NeuronCores,
    weights replicated, zero collectives; outputs concatenated on the host.
  * All state is kept TRANSPOSED on chip ([feature, batch], batch on the free
    axis): gates come out of the PE array as gates^T with the weights as the
    stationary operand, and the elementwise state update directly produces
    h^T, which feeds the next step's matmuls as the moving operand -- the
    recurrence contains no transposes at all.
  * Each core's 128 rows are split into two independent 64-row halves
    executed half-a-step out of phase: while one half runs its sigmoid/tanh +
    c/h update chain on the Scalar/Vector engines, the other half's matmul
    burst keeps the TensorEngine busy.
  * LAGGED-y REFORMULATION: using y_{t-1} = y_{t-2} + h_{t-1} @ W_out.T + b_out,
        gates_t = y_{t-2} @ W_ih.T + h_{t-1} @ (W_hh + W_ih W_out).T
                  + (b_ih + b_hh + W_ih b_out)
    (exact; t=0 is seeded with y_{-1} := y_0 - h_0 @ W_out.T - b_out).  The
    gate matmuls consume a y that was written back a FULL period earlier, so
    the y write-back is never on the critical path.
  * Gate blocks sit in native PyTorch row order [i0..i3|f0..f3|g0..g3|o0..o3]
    and the burst is emitted BLOCK-MAJOR: each block's 5 accumulation matmuls
    (4 W~_hh K-chunks + the [y;1] W_ih chunk) are adjacent, so blocks complete
    progressively; the gates live in THREE separate PSUM tiles (i/f, g, o) so
    each activation only waits for its own blocks (tile-granular deps) and
    the chain starts mid-burst, its tail hiding under the other half's burst.
  * Matmuls run in bf16 (f32 PSUM accumulation); c and y state stay f32;
    activation outputs are bf16 (DVE 2x for i*g and o*tanh(c)).
  * y_t accumulates in a persistent PSUM bank (y_t = y0 + t*b_out + psum);
    the W_out matmuls sit mid-burst so the fp32 y write-back drains in an
    idle DVE slot (never between tanh_g and t1 on the critical chain), and
    the bf16 gate operand y^T is a GpSimd cast-copy of it.  Steady state is
    TensorEngine-bound: ~95% PE occupancy, near-zero inter-matmul gaps.
"""


import os
from contextlib import ExitStack

import ml_dtypes
import numpy as np

import concourse.bass as bass
import concourse.tile as tile
from concourse import bacc, mybir
from concourse.bass_utils import run_bass_kernel_spmd


F32 = mybir.dt.float32
BF16 = mybir.dt.bfloat16
SIG = mybir.ActivationFunctionType.Sigmoid
TANH = mybir.ActivationFunctionType.Tanh

B_TOT = 1024
N_CORES = 8
B = 128          # rows per core
BH = 64          # rows per half
ZD, XD, YD, H = 128, 64, 64, 512
HC, GC = 4, 16   # H chunks of 128; gate blocks of 128 rows

LAST_RESULTS = None
_BUILD_CACHE = {}


def _prep_consts(W_ih, W_hh, b_ih, b_hh, W_proj, b_proj, W_out, b_out):
    bf = ml_dtypes.bfloat16
    # lagged-y reformulation: fold W_ih @ W_out into the recurrent weight and
    # W_ih @ b_out into the gate bias
    Wt = W_hh + W_ih @ W_out               # [2048, 512]
    bt = b_ih + b_hh + W_ih @ b_out        # [2048]

    wg_h = np.empty((128, HC * GC * 128), dtype=bf)
    for k in range(HC):
        for s in range(GC):
            blk = Wt[s * 128:(s + 1) * 128, k * 128:(k + 1) * 128].T  # [K,M]
            wg_h[:, (k * GC + s) * 128:(k * GC + s + 1) * 128] = blk.astype(bf)

    wg_y = np.empty((YD + 1, GC * 128), dtype=bf)
    for s in range(GC):
        wg_y[0:YD, s * 128:(s + 1) * 128] = W_ih[s * 128:(s + 1) * 128, :].T.astype(bf)
        wg_y[YD, s * 128:(s + 1) * 128] = bt[s * 128:(s + 1) * 128].astype(bf)

    wout = np.zeros((128, HC * 128), dtype=bf)
    for k in range(HC):
        wout[:, k * 128:k * 128 + YD] = W_out[:, k * 128:(k + 1) * 128].T.astype(bf)

    wproj = np.empty((ZD, H), dtype=bf)
    for m in range(HC):
        wproj[:, m * 128:(m + 1) * 128] = W_proj[m * 128:(m + 1) * 128, :].T.astype(bf)

    bprojT = b_proj.reshape(HC, 128).T.copy().astype(np.float32)  # [128, HC]
    boutT = b_out.reshape(YD, 1).astype(np.float32)               # [64, 1]
    return dict(wg_h=wg_h, wg_y=wg_y, wout=wout, wproj=wproj,
                bprojT=bprojT, boutT=boutT)


def _build(T):
    nc = bacc.Bacc("TRN2", target_bir_lowering=False, debug=False)

    d_zT = nc.dram_tensor("zT", [ZD, B], BF16, kind="ExternalInput")
    d_y0T = nc.dram_tensor("y0T", [YD, B], F32, kind="ExternalInput")
    d_wg_h = nc.dram_tensor("wg_h", [128, HC * GC * 128], BF16, kind="ExternalInput")
    d_wg_y = nc.dram_tensor("wg_y", [YD + 1, GC * 128], BF16, kind="ExternalInput")
    d_wout = nc.dram_tensor("wout", [128, HC * 128], BF16, kind="ExternalInput")
    d_wproj = nc.dram_tensor("wproj", [ZD, H], BF16, kind="ExternalInput")
    d_bprojT = nc.dram_tensor("bprojT", [128, HC], F32, kind="ExternalInput")
    d_boutT = nc.dram_tensor("boutT", [YD, 1], F32, kind="ExternalInput")
    d_bscan = nc.dram_tensor("bscan", [YD, T], F32, kind="ExternalInput")
    d_out = nc.dram_tensor("out", [YD, T * B], F32, kind="ExternalOutput")

    with ExitStack() as ctx:
        tc = ctx.enter_context(tile.TileContext(nc))
        const = ctx.enter_context(tc.tile_pool(name="const", bufs=1))
        state = ctx.enter_context(tc.tile_pool(name="state", bufs=1))
        actp = ctx.enter_context(tc.tile_pool(name="actp", bufs=4))
        gpsum = ctx.enter_context(tc.tile_pool(name="gpsum", bufs=2, space="PSUM"))
        ypsum = ctx.enter_context(tc.tile_pool(name="ypsum", bufs=1, space="PSUM"))

        wg_h = const.tile([128, HC * GC * 128], BF16)
        wg_y = const.tile([YD + 1, GC * 128], BF16)
        wout = const.tile([128, HC * 128], BF16)
        wproj = const.tile([ZD, H], BF16)
        bprojT = const.tile([128, HC], F32)
        boutT = const.tile([YD, 1], F32)
        bscan = const.tile([YD, T], F32)
        zT = const.tile([ZD, B], BF16)
        y0T = const.tile([YD, B], F32)
        # small tensors first: the init chain (proj matmuls, y_{-1} seed) only
        # needs these and can overlap the 2MB recurrent-weight transfer
        for sb, dr in (
            (wproj, d_wproj), (zT, d_zT), (bprojT, d_bprojT), (boutT, d_boutT),
            (y0T, d_y0T), (bscan, d_bscan), (wout, d_wout), (wg_y, d_wg_y),
            (wg_h, d_wg_h),
        ):
            nc.sync.dma_start(sb[:, :], dr[:, :])

        # per-half state; layout [128, 4*64]: H-chunk j at cols j*64
        cT = [state.tile([128, 256], F32, name=f"cT{h}") for h in range(2)]
        hT = [[state.tile([128, 256], BF16, name=f"hT{p}_{h}") for h in range(2)]
              for p in range(2)]
        yTa = [[state.tile([YD + 1, BH], BF16, name=f"yTa{p}_{h}") for h in range(2)]
               for p in range(2)]
        DMA_CHUNK = 32
        NCH = (T + DMA_CHUNK - 1) // DMA_CHUNK
        # one staging tile per output-DMA chunk: no WAR between later y
        # write-backs and an in-flight chunk DMA (deps are tile-granular)
        ysb = [state.tile([YD, DMA_CHUNK * B], F32, name=f"ysb{c}")
               for c in range(NCH)]
        yp = [ypsum.tile([128, BH], F32, name=f"yp{h}", tag=f"yp{h}")
              for h in range(2)]

        # --- init (both halves): h0 = c0 = proj(z); y_{-1} = y0 - h0@Wout.T - bout
        for h in range(2):
            bsl = slice(h * BH, (h + 1) * BH)
            h0p = gpsum.tile([128, 256], F32, tag="gg", name=f"h0p{h}")
            for m in range(HC):
                nc.tensor.matmul(
                    h0p[:, m * 64:(m + 1) * 64],
                    lhsT=wproj[:, m * 128:(m + 1) * 128],
                    rhs=zT[:, bsl], start=True, stop=True,
                )
            for m in range(HC):
                nc.vector.tensor_scalar_add(
                    cT[h][:, m * 64:(m + 1) * 64],
                    h0p[:, m * 64:(m + 1) * 64],
                    bprojT[:, m:m + 1],
                )
            nc.vector.tensor_copy(hT[1][h][:, :], cT[h][:, :])
            nc.vector.tensor_copy(yTa[1][h][0:YD, :], y0T[:, bsl])
            nc.vector.memset(yTa[0][h][YD:YD + 1, :], 1.0)
            nc.vector.memset(yTa[1][h][YD:YD + 1, :], 1.0)
            # y_{-1} seed
            ypre = gpsum.tile([128, BH], F32, tag="go", name=f"ypre{h}")
            for k in range(HC):
                nc.tensor.matmul(
                    ypre[:, 0:BH],
                    lhsT=wout[:, k * 128:(k + 1) * 128],
                    rhs=hT[1][h][:, k * 64:(k + 1) * 64],
                    start=(k == 0), stop=(k == HC - 1),
                )
            ytmp = actp.tile([YD, BH], F32, tag="ytmp", name=f"ytmp{h}")
            nc.vector.tensor_scalar_add(ytmp[:, :], ypre[0:YD, 0:BH], boutT[:, 0:1])
            nc.vector.tensor_sub(yTa[0][h][0:YD, :], y0T[:, bsl], ytmp[:, :])

        def emit_half(t, h):
            pv = (t + 1) % 2
            cu = t % 2
            bsl = slice(h * BH, (h + 1) * BH)
            # separate PSUM tiles per activation group so each activation
            # only waits for ITS gate blocks (tile-granular dependencies)
            gif = gpsum.tile([128, 512], F32, tag="gif", name=f"gif{t}_{h}")
            gg = gpsum.tile([128, 256], F32, tag="gg", name=f"gg{t}_{h}")
            go = gpsum.tile([128, 256], F32, tag="go", name=f"go{t}_{h}")

            def blk(s):
                if s < 8:
                    return gif[:, s * 64:(s + 1) * 64], s == 0
                if s < 12:
                    return gg[:, (s - 8) * 64:(s - 7) * 64], s == 8
                return go[:, (s - 12) * 64:(s - 11) * 64], s == 12

            # --- block-major gate burst: block s completes at its y-matmul;
            #     the y operand is the LAGGED y_{t-2} (written two bursts ago).
            for s in range(GC):
                out, first = blk(s)
                for k in range(HC):
                    nc.tensor.matmul(
                        out,
                        lhsT=wg_h[:, (k * GC + s) * 128:(k * GC + s + 1) * 128],
                        rhs=hT[pv][h][:, k * 64:(k + 1) * 64],
                        start=(k == 0 and first), stop=False,
                        skip_group_check=True,
                    )
                nc.tensor.matmul(
                    out,
                    lhsT=wg_y[:, s * 128:(s + 1) * 128],
                    rhs=yTa[cu][h][:, :],
                    start=False, stop=True, skip_group_check=True,
                )
                if s == 8 and t > 0:
                    # W_out matmuls mid-burst: the y write-back then becomes
                    # ready early and drains in an idle DVE slot instead of
                    # wedging between tanh_g and t1 on the critical chain
                    for k in range(HC):
                        nc.tensor.matmul(
                            yp[h][:, :],
                            lhsT=wout[:, k * 128:(k + 1) * 128],
                            rhs=hT[pv][h][:, k * 64:(k + 1) * 64],
                            start=(t == 1 and k == 0), stop=False,
                            skip_group_check=True,
                        )

            # --- activation chain; gif: i=[0:256] f=[256:512]; gg: g; go: o
            sg = actp.tile([128, 768], BF16, tag=f"sg{h}", name=f"sg{t}_{h}")
            tg = actp.tile([128, 256], BF16, tag=f"tg{h}", name=f"tg{t}_{h}")
            # sig(i,f) first so the c-update chain starts as early as possible
            nc.scalar.activation(sg[:, 0:512], gif[:, :], SIG)
            t2 = actp.tile([128, 256], F32, tag=f"t2{h}", name=f"t2_{t}_{h}")
            nc.vector.tensor_mul(t2[:, :], sg[:, 256:512], cT[h][:, :])
            nc.scalar.activation(tg[:, :], gg[:, :], TANH)
            t1 = actp.tile([128, 256], BF16, tag=f"t1{h}", name=f"t1_{t}_{h}")
            nc.vector.tensor_mul(t1[:, :], sg[:, 0:256], tg[:, :])
            nc.vector.tensor_add(cT[h][:, :], t2[:, :], t1[:, :])
            nc.scalar.activation(sg[:, 512:768], go[:, :], SIG)
            tch = actp.tile([128, 256], BF16, tag=f"tc{h}", name=f"tc{t}_{h}")
            nc.scalar.activation(tch[:, :], cT[h][:, :], TANH)
            nc.vector.tensor_mul(hT[cu][h][:, :], sg[:, 512:768], tch[:, :])

            # --- y_{t-1} write-back; a full step of slack, keep it off the
            #     critical DVE stretch: one fp32 stt on DVE, then the bf16
            #     gate operand is a GpSimd cast-copy of it.
            if t > 0:
                tp = t - 1
                # y_tp = y0 + psum + (tp+1)*b_out; bias ramp from a host table
                ch, co = tp // DMA_CHUNK, tp % DMA_CHUNK
                sl = ysb[ch][:, co * B + h * BH:co * B + (h + 1) * BH]
                nc.vector.scalar_tensor_tensor(
                    sl, yp[h][0:YD, :], bscan[:, tp:tp + 1],
                    y0T[:, bsl], op0=mybir.AluOpType.add, op1=mybir.AluOpType.add)
                nc.gpsimd.tensor_copy(yTa[tp % 2][h][0:YD, :], sl)
                if h == 1 and co == DMA_CHUNK - 1:
                    lo = ch * DMA_CHUNK * B
                    nc.sync.dma_start(d_out[:, lo:(tp + 1) * B],
                                      ysb[ch][:, 0:(co + 1) * B])

        for t in range(T):
            emit_half(t, 0)
            emit_half(t, 1)

        # final y tails (adds the h_{T-1} @ W_out.T contribution)
        for h in range(2):
            bsl = slice(h * BH, (h + 1) * BH)
            for k in range(HC):
                nc.tensor.matmul(
                    yp[h][:, :],
                    lhsT=wout[:, k * 128:(k + 1) * 128],
                    rhs=hT[(T - 1) % 2][h][:, k * 64:(k + 1) * 64],
                    start=False, stop=(k == HC - 1), skip_group_check=True,
                )
            tp = T - 1
            ch, co = tp // DMA_CHUNK, tp % DMA_CHUNK
            sl = ysb[ch][:, co * B + h * BH:co * B + (h + 1) * BH]
            nc.vector.scalar_tensor_tensor(
                sl, yp[h][0:YD, :], bscan[:, tp:tp + 1],
                y0T[:, bsl], op0=mybir.AluOpType.add, op1=mybir.AluOpType.add)
        ch = (T - 1) // DMA_CHUNK
        lo = ch * DMA_CHUNK * B
        nc.sync.dma_start(d_out[:, lo:T * B], ysb[ch][:, 0:T * B - lo])

    nc.compile()
    return nc


def kernel(z, x, W_ih, W_hh, b_ih, b_hh, W_proj, b_proj, W_out, b_out, y_pred_len):
    global LAST_RESULTS
    z = np.asarray(z, dtype=np.float32)
    x = np.asarray(x, dtype=np.float32)
    T = int(np.asarray(y_pred_len))

    consts = _prep_consts(
        np.asarray(W_ih, np.float32), np.asarray(W_hh, np.float32),
        np.asarray(b_ih, np.float32), np.asarray(b_hh, np.float32),
        np.asarray(W_proj, np.float32), np.asarray(b_proj, np.float32),
        np.asarray(W_out, np.float32), np.asarray(b_out, np.float32),
    )

    if T not in _BUILD_CACHE:
        _BUILD_CACHE[T] = _build(T)
    nc = _BUILD_CACHE[T]
    consts["bscan"] = np.ascontiguousarray(
        np.outer(np.asarray(b_out, np.float32),
                 np.arange(1, T + 1, dtype=np.float32)))

    bf = ml_dtypes.bfloat16
    in_maps = []
    for i in range(N_CORES):
        sl = slice(i * B, (i + 1) * B)
        m = dict(consts)
        m["zT"] = np.ascontiguousarray(z[sl].T.astype(bf))
        m["y0T"] = np.ascontiguousarray(x[sl, -1, :].T.astype(np.float32))
        in_maps.append(m)

    trace = bool(int(os.environ.get("BASS_KERNEL_TRACE", "0")))
    res = run_bass_kernel_spmd(
        nc, in_maps, core_ids=list(range(N_CORES)), trace=trace,
    )
    LAST_RESULTS = res

    outs = [np.ascontiguousarray(
                np.asarray(res.results[i]["out"]).reshape(YD, T, B).transpose(2, 1, 0))
            for i in range(N_CORES)]
    return np.concatenate(outs, axis=0)


# revision 34
# speedup vs baseline: 1.0045x; 1.0045x over previous
"""Trainium2 Bass kernel for nn_Decoder (LSTM decoder with residual output feedback).

Model (per batch row):
    h0 = c0 = z @ W_proj.T + b_proj                      # [B, H]
    y0 = x[:, -1, :]                                     # [B, X]
    per step t: gates = y_{t-1} @ W_ih.T + h_{t-1} @ W_hh.T + (b_ih + b_hh)
                i, f, g, o = split(gates); c = sig(f)*c + sig(i)*tanh(g)
                h = sig(o)*tanh(c); y_t = y_{t-1} + h @ W_out.T + b_out
    out = stack(y_1..y_T)                                # [B, T, Y]

Strategy:
  * Pure data-parallel over batch: B=1024 -> 128 rows/core on 8 NeuronCores,
    weights replicated, zero collectives; outputs concatenated on the host.
  * All state is kept TRANSPOSED on chip ([feature, batch], batch on the free
    axis): gates come out of the PE array as gates^T with the weights as the
    stationary operand, and the elementwise state update directly produces
    h^T, which feeds the next step's matmuls as the moving operand -- the
    recurrence contains no transposes at all.
  * Each core's 128 rows are split into two independent 64-row halves
    executed half-a-step out of phase: while one half runs its sigmoid/tanh +
    c/h update chain on the Scalar/Vector engines, the other half's matmul
    burst keeps the TensorEngine busy.
  * LAGGED-y REFORMULATION: using y_{t-1} = y_{t-2} + h_{t-1} @ W_out.T + b_out,
        gates_t = y_{t-2} @ W_ih.T + h_{t-1} @ (W_hh + W_ih W_out).T
                  + (b_ih + b_hh + W_ih b_out)
    (exact; t=0 is seeded with y_{-1} := y_0 - h_0 @ W_out.T - b_out).  The
    gate matmuls consume a y that was written back a FULL period earlier, so
    the y write-back is never on the critical path.
  * Gate blocks sit in native PyTorch row order [i0..i3|f0..f3|g0..g3|o0..o3]
    and the burst is emitted BLOCK-MAJOR: each block's 5 accumulation matmuls
    (4 W~_hh K-chunks + the [y;1] W_ih chunk) are adjacent, so blocks complete
    progressively; the gates live in THREE separate PSUM tiles (i/f, g, o) so
    each activation only waits for its own blocks (tile-granular deps) and
    the chain starts mid-burst, its tail hiding under the other half's burst.
  * Matmuls run in bf16 (f32 PSUM accumulation); c and y state stay f32;
    activation outputs are bf16 (DVE 2x for i*g and o*tanh(c)).
  * y_t accumulates in a persistent PSUM bank (y_t = y0 + t*b_out + psum);
    the W_out matmuls sit mid-burst so the fp32 y write-back drains in an
    idle DVE slot (never between tanh_g and t1 on the critical chain), and
    the bf16 gate operand y^T is a GpSimd cast-copy of it.  Steady state is
    TensorEngine-bound: ~95% PE occupancy, near-zero inter-matmul gaps.
"""


import os
from contextlib import ExitStack

import ml_dtypes
import numpy as np

import concourse.bass as bass
import concourse.tile as tile
from concourse import bacc, mybir
from concourse.bass_utils import run_bass_kernel_spmd


F32 = mybir.dt.float32
BF16 = mybir.dt.bfloat16
SIG = mybir.ActivationFunctionType.Sigmoid
TANH = mybir.ActivationFunctionType.Tanh

B_TOT = 1024
N_CORES = 8
B = 128          # rows per core
BH = 64          # rows per half
ZD, XD, YD, H = 128, 64, 64, 512
HC, GC = 4, 16   # H chunks of 128; gate blocks of 128 rows

LAST_RESULTS = None
_BUILD_CACHE = {}


def _prep_consts(W_ih, W_hh, b_ih, b_hh, W_proj, b_proj, W_out, b_out):
    bf = ml_dtypes.bfloat16
    # lagged-y reformulation: fold W_ih @ W_out into the recurrent weight and
    # W_ih @ b_out into the gate bias
    Wt = W_hh + W_ih @ W_out               # [2048, 512]
    bt = b_ih + b_hh + W_ih @ b_out        # [2048]

    wg_h = np.empty((128, HC * GC * 128), dtype=bf)
    for k in range(HC):
        for s in range(GC):
            blk = Wt[s * 128:(s + 1) * 128, k * 128:(k + 1) * 128].T  # [K,M]
            wg_h[:, (k * GC + s) * 128:(k * GC + s + 1) * 128] = blk.astype(bf)

    wg_y = np.empty((YD + 1, GC * 128), dtype=bf)
    for s in range(GC):
        wg_y[0:YD, s * 128:(s + 1) * 128] = W_ih[s * 128:(s + 1) * 128, :].T.astype(bf)
        wg_y[YD, s * 128:(s + 1) * 128] = bt[s * 128:(s + 1) * 128].astype(bf)

    wout = np.zeros((128, HC * 128), dtype=bf)
    for k in range(HC):
        wout[:, k * 128:k * 128 + YD] = W_out[:, k * 128:(k + 1) * 128].T.astype(bf)

    wproj = np.empty((ZD, H), dtype=bf)
    for m in range(HC):
        wproj[:, m * 128:(m + 1) * 128] = W_proj[m * 128:(m + 1) * 128, :].T.astype(bf)

    bprojT = b_proj.reshape(HC, 128).T.copy().astype(np.float32)  # [128, HC]
    boutT = b_out.reshape(YD, 1).astype(np.float32)               # [64, 1]
    return dict(wg_h=wg_h, wg_y=wg_y, wout=wout, wproj=wproj,
                bprojT=bprojT, boutT=boutT)


def _build(T):
    nc = bacc.Bacc("TRN2", target_bir_lowering=False, debug=False)

    d_zT = nc.dram_tensor("zT", [ZD, B], BF16, kind="ExternalInput")
    d_y0T = nc.dram_tensor("y0T", [YD, B], F32, kind="ExternalInput")
    d_wg_h = nc.dram_tensor("wg_h", [128, HC * GC * 128], BF16, kind="ExternalInput")
    d_wg_y = nc.dram_tensor("wg_y", [YD + 1, GC * 128], BF16, kind="ExternalInput")
    d_wout = nc.dram_tensor("wout", [128, HC * 128], BF16, kind="ExternalInput")
    d_wproj = nc.dram_tensor("wproj", [ZD, H], BF16, kind="ExternalInput")
    d_bprojT = nc.dram_tensor("bprojT", [128, HC], F32, kind="ExternalInput")
    d_boutT = nc.dram_tensor("boutT", [YD, 1], F32, kind="ExternalInput")
    d_bscan = nc.dram_tensor("bscan", [YD, T], F32, kind="ExternalInput")
    d_out = nc.dram_tensor("out", [YD, T * B], F32, kind="ExternalOutput")

    with ExitStack() as ctx:
        tc = ctx.enter_context(tile.TileContext(nc))
        const = ctx.enter_context(tc.tile_pool(name="const", bufs=1))
        state = ctx.enter_context(tc.tile_pool(name="state", bufs=1))
        actp = ctx.enter_context(tc.tile_pool(name="actp", bufs=4))
        gpsum = ctx.enter_context(tc.tile_pool(name="gpsum", bufs=2, space="PSUM"))
        ypsum = ctx.enter_context(tc.tile_pool(name="ypsum", bufs=1, space="PSUM"))

        wg_h = const.tile([128, HC * GC * 128], BF16)
        wg_y = const.tile([YD + 1, GC * 128], BF16)
        wout = const.tile([128, HC * 128], BF16)
        wproj = const.tile([ZD, H], BF16)
        bprojT = const.tile([128, HC], F32)
        boutT = const.tile([YD, 1], F32)
        bscan = const.tile([YD, T], F32)
        zT = const.tile([ZD, B], BF16)
        y0T = const.tile([YD, B], F32)
        # small tensors first: the init chain (proj matmuls, y_{-1} seed) only
        # needs these and can overlap the 2MB recurrent-weight transfer
        for sb, dr in (
            (wproj, d_wproj), (zT, d_zT), (bprojT, d_bprojT), (boutT, d_boutT),
            (y0T, d_y0T), (bscan, d_bscan), (wout, d_wout), (wg_y, d_wg_y),
            (wg_h, d_wg_h),
        ):
            nc.sync.dma_start(sb[:, :], dr[:, :])

        # per-half state; layout [128, 4*64]: H-chunk j at cols j*64
        cT = [state.tile([128, 256], F32, name=f"cT{h}") for h in range(2)]
        hT = [[state.tile([128, 256], BF16, name=f"hT{p}_{h}") for h in range(2)]
              for p in range(2)]
        yTa = [[state.tile([YD + 1, BH], BF16, name=f"yTa{p}_{h}") for h in range(2)]
               for p in range(2)]
        ysbT = state.tile([YD, T * B], F32)   # out[y, t*128 + h*64 + b]
        yp = [ypsum.tile([128, BH], F32, name=f"yp{h}", tag=f"yp{h}")
              for h in range(2)]

        # --- init (both halves): h0 = c0 = proj(z); y_{-1} = y0 - h0@Wout.T - bout
        for h in range(2):
            bsl = slice(h * BH, (h + 1) * BH)
            h0p = gpsum.tile([128, 256], F32, tag="gg", name=f"h0p{h}")
            for m in range(HC):
                nc.tensor.matmul(
                    h0p[:, m * 64:(m + 1) * 64],
                    lhsT=wproj[:, m * 128:(m + 1) * 128],
                    rhs=zT[:, bsl], start=True, stop=True,
                )
            for m in range(HC):
                nc.vector.tensor_scalar_add(
                    cT[h][:, m * 64:(m + 1) * 64],
                    h0p[:, m * 64:(m + 1) * 64],
                    bprojT[:, m:m + 1],
                )
            nc.vector.tensor_copy(hT[1][h][:, :], cT[h][:, :])
            nc.vector.tensor_copy(yTa[1][h][0:YD, :], y0T[:, bsl])
            nc.vector.memset(yTa[0][h][YD:YD + 1, :], 1.0)
            nc.vector.memset(yTa[1][h][YD:YD + 1, :], 1.0)
            # y_{-1} seed
            ypre = gpsum.tile([128, BH], F32, tag="go", name=f"ypre{h}")
            for k in range(HC):
                nc.tensor.matmul(
                    ypre[:, 0:BH],
                    lhsT=wout[:, k * 128:(k + 1) * 128],
                    rhs=hT[1][h][:, k * 64:(k + 1) * 64],
                    start=(k == 0), stop=(k == HC - 1),
                )
            ytmp = actp.tile([YD, BH], F32, tag="ytmp", name=f"ytmp{h}")
            nc.vector.tensor_scalar_add(ytmp[:, :], ypre[0:YD, 0:BH], boutT[:, 0:1])
            nc.vector.tensor_sub(yTa[0][h][0:YD, :], y0T[:, bsl], ytmp[:, :])

        DMA_CHUNK = 32

        def emit_half(t, h):
            pv = (t + 1) % 2
            cu = t % 2
            bsl = slice(h * BH, (h + 1) * BH)
            # separate PSUM tiles per activation group so each activation
            # only waits for ITS gate blocks (tile-granular dependencies)
            gif = gpsum.tile([128, 512], F32, tag="gif", name=f"gif{t}_{h}")
            gg = gpsum.tile([128, 256], F32, tag="gg", name=f"gg{t}_{h}")
            go = gpsum.tile([128, 256], F32, tag="go", name=f"go{t}_{h}")

            def blk(s):
                if s < 8:
                    return gif[:, s * 64:(s + 1) * 64], s == 0
                if s < 12:
                    return gg[:, (s - 8) * 64:(s - 7) * 64], s == 8
                return go[:, (s - 12) * 64:(s - 11) * 64], s == 12

            # --- block-major gate burst: block s completes at its y-matmul;
            #     the y operand is the LAGGED y_{t-2} (written two bursts ago).
            for s in range(GC):
                out, first = blk(s)
                for k in range(HC):
                    nc.tensor.matmul(
                        out,
                        lhsT=wg_h[:, (k * GC + s) * 128:(k * GC + s + 1) * 128],
                        rhs=hT[pv][h][:, k * 64:(k + 1) * 64],
                        start=(k == 0 and first), stop=False,
                        skip_group_check=True,
                    )
                nc.tensor.matmul(
                    out,
                    lhsT=wg_y[:, s * 128:(s + 1) * 128],
                    rhs=yTa[cu][h][:, :],
                    start=False, stop=True, skip_group_check=True,
                )
                if s == 6 and t > 0:
                    # W_out matmuls mid-burst: the y write-back then becomes
                    # ready early and drains in an idle DVE slot instead of
                    # wedging between tanh_g and t1 on the critical chain
                    for k in range(HC):
                        nc.tensor.matmul(
                            yp[h][:, :],
                            lhsT=wout[:, k * 128:(k + 1) * 128],
                            rhs=hT[pv][h][:, k * 64:(k + 1) * 64],
                            start=(t == 1 and k == 0), stop=False,
                            skip_group_check=True,
                        )

            # --- activation chain; gif: i=[0:256] f=[256:512]; gg: g; go: o
            sg = actp.tile([128, 768], BF16, tag=f"sg{h}", name=f"sg{t}_{h}")
            tg = actp.tile([128, 256], BF16, tag=f"tg{h}", name=f"tg{t}_{h}")
            # sig(i,f) first so the c-update chain starts as early as possible
            nc.scalar.activation(sg[:, 0:512], gif[:, :], SIG)
            t2 = actp.tile([128, 256], F32, tag=f"t2{h}", name=f"t2_{t}_{h}")
            nc.vector.tensor_mul(t2[:, :], sg[:, 256:512], cT[h][:, :])
            nc.scalar.activation(tg[:, :], gg[:, :], TANH)
            t1 = actp.tile([128, 256], BF16, tag=f"t1{h}", name=f"t1_{t}_{h}")
            nc.vector.tensor_mul(t1[:, :], sg[:, 0:256], tg[:, :])
            nc.vector.tensor_add(cT[h][:, :], t2[:, :], t1[:, :])
            nc.scalar.activation(sg[:, 512:768], go[:, :], SIG)
            tch = actp.tile([128, 256], BF16, tag=f"tc{h}", name=f"tc{t}_{h}")
            nc.scalar.activation(tch[:, :], cT[h][:, :], TANH)
            nc.vector.tensor_mul(hT[cu][h][:, :], sg[:, 512:768], tch[:, :])

            # --- y_{t-1} write-back; a full step of slack, keep it off the
            #     critical DVE stretch: one fp32 stt on DVE, then the bf16
            #     gate operand is a GpSimd cast-copy of it.
            if t > 0:
                tp = t - 1
                # y_tp = y0 + psum + (tp+1)*b_out; bias ramp from a host table
                sl = ysbT[:, tp * B + h * BH:tp * B + (h + 1) * BH]
                nc.vector.scalar_tensor_tensor(
                    sl, yp[h][0:YD, :], bscan[:, tp:tp + 1],
                    y0T[:, bsl], op0=mybir.AluOpType.add, op1=mybir.AluOpType.add)
                nc.gpsimd.tensor_copy(yTa[tp % 2][h][0:YD, :], sl)
                if h == 1 and (tp % DMA_CHUNK == DMA_CHUNK - 1):
                    lo = (tp // DMA_CHUNK) * DMA_CHUNK * B
                    nc.sync.dma_start(d_out[:, lo:(tp + 1) * B],
                                      ysbT[:, lo:(tp + 1) * B])

        for t in range(T):
            emit_half(t, 0)
            emit_half(t, 1)

        # final y tails (adds the h_{T-1} @ W_out.T contribution)
        for h in range(2):
            bsl = slice(h * BH, (h + 1) * BH)
            for k in range(HC):
                nc.tensor.matmul(
                    yp[h][:, :],
                    lhsT=wout[:, k * 128:(k + 1) * 128],
                    rhs=hT[(T - 1) % 2][h][:, k * 64:(k + 1) * 64],
                    start=False, stop=(k == HC - 1), skip_group_check=True,
                )
            tp = T - 1
            sl = ysbT[:, tp * B + h * BH:tp * B + (h + 1) * BH]
            nc.vector.scalar_tensor_tensor(
                sl, yp[h][0:YD, :], bscan[:, tp:tp + 1],
                y0T[:, bsl], op0=mybir.AluOpType.add, op1=mybir.AluOpType.add)
        lo = ((T - 1) // DMA_CHUNK) * DMA_CHUNK * B
        nc.sync.dma_start(d_out[:, lo:T * B], ysbT[:, lo:T * B])

    nc.compile()
    return nc


def kernel(z, x, W_ih, W_hh, b_ih, b_hh, W_proj, b_proj, W_out, b_out, y_pred_len):
    global LAST_RESULTS
    z = np.asarray(z, dtype=np.float32)
    x = np.asarray(x, dtype=np.float32)
    T = int(np.asarray(y_pred_len))

    consts = _prep_consts(
        np.asarray(W_ih, np.float32), np.asarray(W_hh, np.float32),
        np.asarray(b_ih, np.float32), np.asarray(b_hh, np.float32),
        np.asarray(W_proj, np.float32), np.asarray(b_proj, np.float32),
        np.asarray(W_out, np.float32), np.asarray(b_out, np.float32),
    )

    if T not in _BUILD_CACHE:
        _BUILD_CACHE[T] = _build(T)
    nc = _BUILD_CACHE[T]
    consts["bscan"] = np.ascontiguousarray(
        np.outer(np.asarray(b_out, np.float32),
                 np.arange(1, T + 1, dtype=np.float32)))

    bf = ml_dtypes.bfloat16
    in_maps = []
    for i in range(N_CORES):
        sl = slice(i * B, (i + 1) * B)
        m = dict(consts)
        m["zT"] = np.ascontiguousarray(z[sl].T.astype(bf))
        m["y0T"] = np.ascontiguousarray(x[sl, -1, :].T.astype(np.float32))
        in_maps.append(m)

    trace = bool(int(os.environ.get("BASS_KERNEL_TRACE", "0")))
    res = run_bass_kernel_spmd(
        nc, in_maps, core_ids=list(range(N_CORES)), trace=trace,
    )
    LAST_RESULTS = res

    outs = [np.ascontiguousarray(
                np.asarray(res.results[i]["out"]).reshape(YD, T, B).transpose(2, 1, 0))
            for i in range(N_CORES)]
    return np.concatenate(outs, axis=0)
